# revision 57
# baseline (speedup 1.0000x reference)
"""GCN encoder (5-layer GCNConv + global mean pool) on 8 Trainium2 NeuronCores.

Strategy (node sharding, v3):
  - 10000 nodes split contiguously across 8 cores (1250/core, padded 1280).
  - REAL edges bucketed by (dst core, dst tile of 128, src half); src half
    = first/second 640 local rows of the src's core. One shared Seg (GCN
    norm folded into one-hot values) + gather-index table drives the
    aggregation matmuls of ALL 5 layers. Buckets are ragged (per-bucket
    chunk counts); idx slots beyond the max-core valid count are -1
    (SWDGE skips them; pool buffers are zero-warmed once so stale tails
    are finite and killed by the zero Seg columns).
  - Self-loops are NOT gathered: each h0-bucket's aggregation chain ends
    with a diagonal-stationary matmul applied to the locally resident
    message tile (xloc for L1, the hres h@W tile for L2-5) — saves ~6%
    of descriptors/bytes per layer.
  - Layer 1's messages (a pure reindex of the static input x) are
    pregathered on host and streamed by static HWDGE DMA — no SWDGE at
    all in L1, which hides the startup barrier + first-collective warmup
    (absorbed by a dummy AllGather).
  - Layers 2-5: per-tile GEMM (h @ W, fp16, PSUM fp32) -> fp16 cast into
    hres (ACT) -> per-half bounce -> per-half AllGather into a 5120-row
    shared table -> SWDGE gathers (4-way queue split per bucket; SWDGE
    desc-gen at ~4.2ns/desc is the pacing resource) -> one-hot matmul
    aggregation. AG(h0) triggers at h1-t7 (POST(t4) bounce has landed,
    CC stream free); AG(h1) at layer end hides under the next h0 pass.
    Bias folds into the h0 partial copy (DVE); relu+cast runs on ACT;
    transposes on PE.
  - Mean-pool as matmul with 1/count one-hot, AllReduce over cores.

Graph structure (edge sort, Seg with norm, gather indices, pregathered x
messages, pool matrix) is preprocessed on host; all FLOPs on x/W run on
device.
"""
import sys

import numpy as np

sys.path.insert(0, "/opt/trn_rl_repo")

import concourse.bacc as bacc
import concourse.bass as bass  # noqa: F401
import concourse.mybir as mybir
import concourse.tile as tile
from concourse import bass_utils

dt = mybir.dt
AF = mybir.ActivationFunctionType

N = 10000
E = 150000
G = 64
C = 8
DIN = 128
DHID = 512
DOUT = 128
NPC = N // C          # 1250 nodes per core
NTILE = 10            # tiles of 128 dst nodes per core
HROWS = C * 640       # 5120 rows per half table
FO = [DHID, DHID, DHID, DHID, DOUT]   # per-layer GEMM output widths
FIT = [1, 4, 4, 4, 4]                 # fi tiles per GEMM


def _pregather_x(x, gidx, meta):
    """Host-side L1 message tiles: the layer-1 gather reads the STATIC input
    x, so its gather output (a pure reindex of x, same class of host prep as
    gidx/seg) is precomputed and streamed in by static DMA — no SWDGE."""
    kp, cbase, mreg, nchunk = meta
    h0 = np.zeros((HROWS, DIN), np.float16)
    h1 = np.zeros((HROWS, DIN), np.float16)
    for c in range(C):
        h0[c * 640:(c + 1) * 640] = x[c * NPC: c * NPC + 640]
        h1[c * 640: c * 640 + NPC - 640] = x[c * NPC + 640:(c + 1) * NPC]
    ht = [h0, h1]
    xg = np.zeros((C, 128, nchunk, 128), np.float16)
    for c in range(C):
        for b in range(2 * NTILE):
            h = b // NTILE
            for k in range(kp[b]):
                idx = gidx[c, (cbase[b] + k) * 128:(cbase[b] + k + 1) * 128]
                valid = idx >= 0
                # out[p, k, :] = table[idx[k*128+p], :]; -1 slots stay 0
                xg[c, valid, cbase[b] + k, :] = ht[h][idx[valid]]
    return xg


def _preprocess(edge_index, batch):
    """Bucket REAL edges by (src half, dst tile) per dst core; build Seg,
    gather indices (half-local rows, -1 padded), and the pool matrix. The
    synthetic self-loops are NOT bucketed: their messages are the locally
    resident h@W tiles, added on-device via a diagonal-stationary matmul
    (saves ~6% of gather descriptors/bytes per layer)."""
    src = edge_index[0].astype(np.int64)
    dst = edge_index[1].astype(np.int64)
    deg = np.bincount(dst, minlength=N).astype(np.float64) + 1.0  # + self-loop
    dinv = 1.0 / np.sqrt(deg)
    norm = (dinv[src] * dinv[dst]).astype(np.float32)
    selfw = (dinv * dinv).astype(np.float32)  # self-loop weight per node

    core = dst // NPC
    t_of = (dst % NPC) // 128
    dloc = (dst % NPC) % 128

    s_core = src // NPC
    s_loc = src % NPC
    s_half = (s_loc >= 640).astype(np.int64)
    s_row = s_core * 640 + (s_loc - s_half * 640)   # row within half table

    # bucket key, h-major: (src half, dst tile) within each dst core.
    # Duplicate src rows within a bucket share one gather slot (the Seg
    # column for that slot is multi-hot with summed norms).
    hb = s_half * NTILE + t_of          # 0..19
    order = np.lexsort((s_row, hb, core))
    b_row, b_hb, b_core, b_dloc, b_norm = (
        s_row[order], hb[order], core[order], dloc[order], norm[order])
    gbucket = b_core * (2 * NTILE) + b_hb
    new_grp = np.ones(len(order), bool)
    new_grp[1:] = (gbucket[1:] != gbucket[:-1]) | (b_row[1:] != b_row[:-1])
    uid = np.cumsum(new_grp) - 1
    bnd = np.ones(len(order), bool)
    bnd[1:] = gbucket[1:] != gbucket[:-1]
    start_uid = np.maximum.accumulate(np.where(bnd, uid, -1))
    slot = uid - start_uid

    ucnt = np.zeros((C, 2 * NTILE), np.int64)
    np.add.at(ucnt, (b_core, b_hb), new_grp)
    mreg = ucnt.max(axis=0)             # uniform valid slot count per bucket
    kp = np.maximum((mreg + 127) // 128, 1)   # chunks per bucket
    base = np.zeros(2 * NTILE, np.int64)
    base[1:] = np.cumsum(kp)[:-1]
    nchunk = int(kp.sum())

    chunk = base[b_hb] + slot // 128
    erow = slot % 128

    gidx = np.full((C, nchunk * 128), -1, np.int16)
    seg32 = np.zeros((C, 128, nchunk, 128), np.float32)
    gidx[b_core, chunk * 128 + erow] = b_row.astype(np.int16)
    np.add.at(seg32, (b_core, erow, chunk, b_dloc), b_norm)
    seg = seg32.astype(np.float16)
    # pad every bucket to the uniform valid count with idx-0 fillers
    for c in range(C):
        for b in range(2 * NTILE):
            lo, hi = base[b] * 128 + ucnt[c, b], base[b] * 128 + mreg[b]
            gidx[c, lo:hi] = 0

    # idx wrap: logical idx i -> partition i%16, column i//16; replicate x8
    gidx_w = np.ascontiguousarray(
        np.tile(gidx.reshape(C, -1, 16).transpose(0, 2, 1), (1, 8, 1)))

    # pool matrix [C, 128, NTILE, G]: 1/count at (node row, graph)
    gcnt = np.bincount(batch, minlength=G).astype(np.float64)
    inv = (1.0 / np.maximum(gcnt, 1.0))
    pool = np.zeros((C, 128, NTILE, G), np.float16)
    nodes = np.arange(N)
    pc, pr = nodes // NPC, nodes % NPC
    pool[pc, pr % 128, pr // 128, batch] = inv[batch].astype(np.float16)

    # self-loop diagonal stationary [C, 128, NTILE, 128]
    selfd = np.zeros((C, 128, NTILE, 128), np.float16)
    selfd[pc, pr % 128, pr // 128, pr % 128] = selfw[nodes]

    meta = (tuple(int(v) for v in kp), tuple(int(v) for v in base),
            tuple(int(v) for v in mreg), nchunk)
    return gidx_w, gidx, seg, pool, selfd, meta


def _build(meta):
    kp, cbase, mreg, nchunk = meta
    kpmax = max(kp)
    nc = bacc.Bacc("TRN2", target_bir_lowering=False, debug=False,
                   num_devices=C, num_swdge_queues=4)

    xg_in = nc.dram_tensor("xg_in", [128, nchunk, DIN], dt.float16,
                           kind="ExternalInput")
    w_in = [nc.dram_tensor(f"w{i}_in", [DIN if i == 0 else DHID, FO[i]],
                           dt.float32, kind="ExternalInput") for i in range(5)]
    b_in = [nc.dram_tensor(f"b{i}_in", [128, FO[i]], dt.float32,
                           kind="ExternalInput") for i in range(5)]
    seg_in = nc.dram_tensor("seg_in", [128, nchunk, 128], dt.float16,
                            kind="ExternalInput")
    gidx_in = nc.dram_tensor("gidx_in", [128, nchunk * 8], dt.int16,
                             kind="ExternalInput")
    pool_in = nc.dram_tensor("pool_in", [128, NTILE, G], dt.float16,
                             kind="ExternalInput")
    selfd_in = nc.dram_tensor("selfd_in", [128, NTILE, 128], dt.float16,
                              kind="ExternalInput")
    xloc_in = nc.dram_tensor("xloc_in", [128, NTILE, DIN], dt.float16,
                             kind="ExternalInput")
    id_in = nc.dram_tensor("id_in", [128, 128], dt.float16, kind="ExternalInput")
    out = nc.dram_tensor("out", [G, DOUT], dt.float32, kind="ExternalOutput")
    ccw_in = nc.dram_tensor("ccw_in", [1, 16], dt.float16)
    ccw_out = nc.dram_tensor("ccw_out", [C, 16], dt.float16, addr_space="Shared")

    gshA = [nc.dram_tensor(f"gshA{h}", [HROWS, DHID], dt.float16,
                           addr_space="Shared") for h in range(2)]
    gshB = [nc.dram_tensor(f"gshB{h}", [HROWS, DOUT], dt.float16,
                           addr_space="Shared") for h in range(2)]
    bounceA = [nc.dram_tensor(f"bounceA{h}", [640, DHID], dt.float16)
               for h in range(2)]
    bounceB = [nc.dram_tensor(f"bounceB{h}", [640, DOUT], dt.float16)
               for h in range(2)]
    pool_sh = nc.dram_tensor("pool_sh", [G, DOUT], dt.float32, addr_space="Shared")
    pool_bounce = nc.dram_tensor("pool_bounce", [G, DOUT], dt.float32)

    with tile.TileContext(nc) as tc:
        with (
            tc.tile_pool(name="const", bufs=1) as cp,
            tc.tile_pool(name="work", bufs=2) as wp,
            tc.tile_pool(name="msgp", bufs=6) as mp,
            tc.tile_pool(name="gemm_ps", bufs=2, space="PSUM") as gps,
            tc.tile_pool(name="agg_ps", bufs=2, space="PSUM") as aps,
            tc.tile_pool(name="tp_ps", bufs=2, space="PSUM") as tps,
            tc.tile_pool(name="pool_ps", bufs=1, space="PSUM") as pps,
        ):
            # ---- resident tensors. Only what L1's first buckets need loads
            # up front (self tiles + first seg quarter); the rest streams in
            # from inside L1's bucket loop so the sync-DMA FIFO never delays
            # the L1 message tiles.
            gidx_sb = cp.tile([128, nchunk * 8], dt.int16)
            seg_sb = cp.tile([128, nchunk, 128], dt.float16)
            scut = [0, cbase[5], cbase[NTILE], cbase[NTILE + 5], nchunk]

            def load_seg(si):
                nc.sync.dma_start(
                    out=seg_sb[:, scut[si]:scut[si + 1], :],
                    in_=seg_in[:, scut[si]:scut[si + 1], :])

            pool_sb = cp.tile([128, NTILE, G], dt.float16)
            id16 = cp.tile([128, 128], dt.float16)
            breps = cp.tile([128, 4, DHID], dt.float32)
            brep5 = cp.tile([128, DOUT], dt.float32)

            # weights -> fp16 tiles. slots: W1 -> w16[:,0]; W2..W4 -> 1+4(i-1)+j
            w16 = cp.tile([128, 13, DHID], dt.float16)
            w516 = cp.tile([128, 4, DOUT], dt.float16)

            def load_weight(i):
                for j in range(FIT[i]):
                    wstage = wp.tile([128, FO[i]], dt.float32, tag="wstage")
                    nc.sync.dma_start(
                        out=wstage[:, :], in_=w_in[i][j * 128:(j + 1) * 128, :])
                    if i < 4:
                        nc.vector.tensor_copy(
                            w16[:, (0 if i == 0 else 1 + 4 * (i - 1)) + j, :],
                            wstage[:, :])
                    else:
                        nc.vector.tensor_copy(w516[:, j, :], wstage[:, :])

            hT = cp.tile([128, NTILE, 4, 128], dt.float16)
            h_out = cp.tile([128, NTILE, DOUT], dt.float16)
            partial = cp.tile([128, NTILE, DHID], dt.float16)
            hres = cp.tile([128, NTILE, DHID], dt.float16)
            selfd_sb = cp.tile([128, NTILE, 128], dt.float16)
            nc.sync.dma_start(out=selfd_sb[:, :, :], in_=selfd_in[:, :, :])
            xloc_sb = cp.tile([128, NTILE, DIN], dt.float16)
            nc.sync.dma_start(out=xloc_sb[:, :, :], in_=xloc_in[:, :, :])
            load_seg(0)
            nc.sync.dma_start(out=id16[:, :], in_=id_in[:, :])
            pp = pps.tile([64, DOUT], dt.float32)

            # absorb the ~11us first-collective setup cost during L1 with a
            # dummy 32-byte AllGather (values unused)
            nc.gpsimd.collective_compute(
                "AllGather", mybir.AluOpType.bypass,
                replica_groups=[list(range(C))],
                ins=[ccw_in.ap().opt()], outs=[ccw_out.ap().opt()])

            def warm_msg_pool(i):
                """One message-pool buffer memset, interleaved into L1's h1
                pass (DVE slack there): the gathers skip -1 tail slots, so
                the buffers must start finite — stale data from reuse is
                harmless (the Seg columns there are 0), initial NaN is not
                (0*NaN=NaN)."""
                if i % 2 == 0:
                    w = mp.tile([128, kpmax, DHID], dt.float16, tag="mA")
                else:
                    w = mp.tile([128, kpmax, DOUT], dt.float16, tag="mB")
                nc.vector.memset(w[:, :, :].rearrange("p a b -> p (a b)"), 0.0)

            def gather_bucket(l, h, t):
                """Fetch bucket (h,t)'s message tile. Layer 1: static HWDGE
                stream of the host-pregathered x messages. Layers 2-5: SWDGE
                gather split across all 4 queues (desc generation is the
                pacing resource — split to overlap gen with transfer)."""
                b = h * NTILE + t
                bi = (l - 1) * 2 * NTILE + b
                if l == 1:
                    mtile = mp.tile([128, kpmax, DIN], dt.float16, tag="m16")
                    nc.sync.dma_start(
                        out=mtile[:, 0:kp[b], :],
                        in_=xg_in[:, cbase[b]:cbase[b] + kp[b], :])
                    return mtile
                if l == 5:
                    mtile = mp.tile([128, kpmax, DOUT], dt.float16, tag="mB")
                    src, esz = gshB[h], DOUT
                else:
                    mtile = mp.tile([128, kpmax, DHID], dt.float16, tag="mA")
                    src, esz = gshA[h], DHID
                # split chunks ~evenly across the 4 SWDGE queues
                kq, rq = divmod(kp[b], 4)
                splits, k0 = [], 0
                for s in range(4):
                    k1 = k0 + kq + (1 if s < rq else 0)
                    if k1 > k0:
                        splits.append((s, k0, k1))
                    k0 = k1
                for s, k0, k1 in splits:
                    nreg = min(mreg[b], k1 * 128) - min(mreg[b], k0 * 128)
                    if nreg <= 0:
                        continue
                    nc.gpsimd.dma_gather(
                        out_ap=mtile[:, k0:k1, :],
                        in_ap=src[:, :],
                        idxs_ap=gidx_sb[:, (cbase[b] + k0) * 8:
                                        (cbase[b] + k1) * 8],
                        num_idxs=(k1 - k0) * 128,
                        num_idxs_reg=nreg,
                        elem_size=esz,
                        single_packet=False,
                        queue_num=(bi + s) % 4,
                    )
                return mtile

            def agg_bucket(l, h, t, mtile):
                """One-hot matmul accumulation of bucket (h,t) into a fresh
                PSUM aggregator; returns the aggregator tile. The h0 chain
                also adds the self-loop term: a diagonal stationary applied
                to the locally resident message tile (x for L1, h@W for
                L2-5) — no gather needed for self edges."""
                b = h * NTILE + t
                fo = DIN if l == 1 else FO[l - 1]
                pa = aps.tile([128, DHID], dt.float32, tag="pa")
                for k in range(kp[b]):
                    nc.tensor.matmul(
                        pa[:, :fo], seg_sb[:, cbase[b] + k, :], mtile[:, k, :],
                        start=(k == 0),
                        stop=(h == 1 and k == kp[b] - 1))
                if h == 0:
                    sm = xloc_sb[:, t, :] if l == 1 else hres[:, t, :fo]
                    nc.tensor.matmul(pa[:, :fo], selfd_sb[:, t, :], sm,
                                     start=False, stop=True)
                return pa

            def post_tile(l, t):
                """After both halves of tile t are aggregated for layer l:
                finish the tile and stage the next layer's table."""
                fo = DIN if l == 1 else FO[l - 1]
                hsum = wp.tile([128, fo], dt.float32, tag=f"hsum{fo}")
                nc.vector.tensor_tensor(
                    hsum[:, :], post_tile.pa[:, :fo], partial[:, t, :fo],
                    mybir.AluOpType.add)
                if l == 1:
                    # (Ax) -> fp16 -> transpose -> @W1 + b1 -> relu
                    st16 = wp.tile([128, DIN], dt.float16, tag="st16")
                    nc.scalar.activation(st16[:, :], hsum[:, :], AF.Copy)
                    pt1 = tps.tile([128, DHID], dt.float16, tag="pt")
                    nc.tensor.transpose(pt1[:, :128], st16[:, :], id16[:, :])
                    xT = wp.tile([128, DIN], dt.float16, tag="xT")
                    nc.vector.tensor_copy(xT[:, :], pt1[:, :128])
                    pg = gps.tile([128, DHID], dt.float32, tag="pg")
                    nc.tensor.matmul(pg[:, :], xT[:, :], w16[:, 0, :],
                                     start=True, stop=True)
                    hs2 = wp.tile([128, DHID], dt.float32, tag="hsum512")
                    nc.vector.tensor_tensor(
                        hs2[:, :], pg[:, :], breps[:, 0, :],
                        mybir.AluOpType.add)
                    hnm = wp.tile([128, DHID], dt.float16, tag="hnm")
                    nc.scalar.activation(hnm[:, :], hs2[:, :], AF.Relu)
                elif l < 5:
                    hnm = wp.tile([128, fo], dt.float16, tag="hnm")
                    nc.scalar.activation(hnm[:, :], hsum[:, :], AF.Relu)
                else:
                    nc.scalar.activation(h_out[:, t, :], hsum[:, :], AF.Relu)
                    nc.tensor.matmul(
                        pp[:, :], pool_sb[:, t, :64], h_out[:, t, :],
                        start=(t == 0), stop=(t == NTILE - 1))
                    return

                # transposes -> hT -> GEMM W_{l+1} -> cast -> bounce half
                fon = FO[l]
                bounce = bounceA if l < 4 else bounceB
                pt = tps.tile([128, DHID], dt.float16, tag="pt")
                for j in range(4):
                    nc.tensor.transpose(
                        pt[:, j * 128:(j + 1) * 128],
                        hnm[:, j * 128:(j + 1) * 128], id16[:, :])
                nc.vector.tensor_copy(
                    hT[:, t, :, :].rearrange("p a b -> p (a b)"), pt[:, :512])
                pg2 = gps.tile([128, fon], dt.float32, tag="pg")
                for j in range(4):
                    wslot = (w16[:, 1 + 4 * (l - 1) + j, :] if l < 4
                             else w516[:, j, :])
                    nc.tensor.matmul(pg2[:, :], hT[:, t, j, :], wslot,
                                     start=(j == 0), stop=(j == 3))
                # h@W lands in the resident hres slot: it is both the bounce
                # source and the next layer's self-loop message tile
                nc.scalar.activation(hres[:, t, :fon], pg2[:, :], AF.Copy)
                hh, r = t // 5, (t % 5) * 128
                nc.sync.dma_start(out=bounce[hh].ap()[r:r + 128, :],
                                  in_=hres[:, t, :fon])

            def ag_half(bounce, gsh):
                nc.gpsimd.collective_compute(
                    "AllGather", mybir.AluOpType.bypass,
                    replica_groups=[list(range(C))],
                    ins=[bounce.ap().opt()],
                    outs=[gsh.ap().opt()])

            # ========================= LAYERS 1..5 =========================
            # Pool-stream order per layer: [AG(l,h0) trigger] h0 gathers,
            # [AG(l,h1) trigger] h1 gathers. The AG triggers wait on the
            # previous layer's bounce writes; placing them at half-pass
            # heads keeps them from head-of-line-blocking gather issue.
            # post_tile lags the aggregation by one bucket so its
            # DVE->ACT->PE chain hides under the next bucket's matmuls.
            for l in range(1, 6):
                fo = DIN if l == 1 else FO[l - 1]
                # deferred const loads ride this layer's AllGather gap
                # (W_{l+1}/b_{l} are first needed during layer l's passes)
                if l == 2:
                    load_weight(2)
                    nc.sync.dma_start(out=breps[:, 2, :], in_=b_in[2][:, :])
                elif l == 3:
                    load_weight(3)
                    nc.sync.dma_start(out=breps[:, 3, :], in_=b_in[3][:, :])
                elif l == 4:
                    load_weight(4)
                    nc.sync.dma_start(out=brep5[:, :], in_=b_in[4][:, :])
                    nc.sync.dma_start(out=pool_sb[:, :, :], in_=pool_in[:, :, :])
                for h in range(2):
                    pend = None
                    for t in range(NTILE):
                        # L1: stream the remaining residents between this
                        # layer's message tiles, each a few buckets before
                        # its first consumer
                        if l == 1 and h == 0:
                            if t == 2:
                                load_seg(1)
                            elif t == 3:
                                nc.sync.dma_start(out=breps[:, 0, :],
                                                  in_=b_in[0][:, :])
                                nc.sync.dma_start(out=breps[:, 1, :],
                                                  in_=b_in[1][:, :])
                                load_weight(0)
                            elif t == 6:
                                load_seg(2)
                            elif t == 7:
                                load_weight(1)
                            elif t == 8:
                                nc.sync.dma_start(out=gidx_sb[:, :],
                                                  in_=gidx_in[:, :])
                        if l == 1 and h == 1:
                            if t == 1:
                                load_seg(3)
                            warm_msg_pool(t)
                            if t < 2:
                                warm_msg_pool(t + 10)
                        if l < 5 and h == 1 and t == 6:
                            # trigger the next layer's h0 table AllGather:
                            # POST(t4) flushed WITHOUT the usual one-bucket
                            # lag (below), so its bounce landed during t5,
                            # this layer's h0-table reads ended at the
                            # half-pass, and the CC stream is free — the AG
                            # gets ~2 extra buckets of overlap with the h1
                            # tail
                            ag_half((bounceA if l < 4 else bounceB)[0],
                                    (gshA if l < 4 else gshB)[0])
                        mt = gather_bucket(l, h, t)
                        pa = agg_bucket(l, h, t, mt)
                        if h == 0:
                            # fold the post-agg bias into the partial copy
                            if l == 1:
                                nc.vector.tensor_copy(
                                    partial[:, t, :fo], pa[:, :fo])
                            else:
                                nc.vector.tensor_tensor(
                                    partial[:, t, :fo], pa[:, :fo],
                                    breps[:, l - 1, :fo] if l < 5
                                    else brep5[:, :],
                                    mybir.AluOpType.add)
                        else:
                            if pend is not None:
                                post_tile.pa = pend[1]
                                post_tile(l, pend[0])
                            if t == 4:
                                # flush t4 immediately (it gates the next
                                # layer's h0 AllGather trigger at t6)
                                post_tile.pa = pa
                                post_tile(l, 4)
                                pend = None
                            else:
                                pend = (t, pa)
                    if h == 1:
                        post_tile.pa = pend[1]
                        post_tile(l, pend[0])
                        if l < 5:
                            # h1 table AllGather: emitted after POST(l,t9)
                            # exists; hides under AG(h0) + the h0 gather pass
                            ag_half((bounceA if l < 4 else bounceB)[1],
                                    (gshA if l < 4 else gshB)[1])

            # ---- mean pool: AllReduce over cores ----
            pres = wp.tile([64, DOUT], dt.float32, tag="pres")
            nc.vector.tensor_copy(pres[:, :], pp[:, :])
            nc.sync.dma_start(out=pool_bounce[:, :], in_=pres[:, :])
            nc.gpsimd.collective_compute(
                "AllReduce", mybir.AluOpType.add,
                replica_groups=[list(range(C))],
                ins=[pool_bounce.ap().opt()],
                outs=[pool_sh.ap().opt()])
            ores = wp.tile([64, DOUT], dt.float32, tag="ores")
            nc.sync.dma_start(out=ores[:, :], in_=pool_sh[:, :])
            nc.sync.dma_start(out=out[:, :], in_=ores[:, :])

    nc.compile()
    return nc


_CACHE = {}


def _get_program(meta):
    if meta not in _CACHE:
        _CACHE[meta] = _build(meta)
    return _CACHE[meta]


def make_in_maps(inputs):
    edge_index = np.asarray(inputs["edge_index"])
    batch = np.asarray(inputs["batch"])
    x = np.asarray(inputs["x"], dtype=np.float32)
    gidx_w, gidx, seg, pool, selfd, meta = _preprocess(edge_index, batch)
    xg = _pregather_x(x, gidx, meta)
    x16 = x.astype(np.float16)
    xloc = np.zeros((C, 128, NTILE, DIN), np.float16)
    for c in range(C):
        for t in range(NTILE):
            lo = c * NPC + t * 128
            nrow = min(128, NPC - t * 128)
            xloc[c, :nrow, t, :] = x16[lo:lo + nrow]
    ident = np.eye(128, dtype=np.float16)
    in_maps = []
    for c in range(C):
        m = {
            "xg_in": np.ascontiguousarray(xg[c]),
            "seg_in": np.ascontiguousarray(seg[c]),
            "gidx_in": gidx_w[c],
            "pool_in": np.ascontiguousarray(pool[c]),
            "selfd_in": np.ascontiguousarray(selfd[c]),
            "xloc_in": np.ascontiguousarray(xloc[c]),
            "id_in": ident,
        }
        for i in range(5):
            w = np.asarray(inputs[f"W{i + 1}"], dtype=np.float32)
            b = np.asarray(inputs[f"b{i + 1}"], dtype=np.float32)
            m[f"w{i}_in"] = w
            m[f"b{i}_in"] = np.ascontiguousarray(np.tile(b[None, :], (128, 1)))
        in_maps.append(m)
    return in_maps, meta


def kernel(**inputs):
    in_maps, meta = make_in_maps(inputs)
    nc = _get_program(meta)
    res = bass_utils.run_bass_kernel_spmd(
        nc, in_maps, core_ids=list(range(C)))
    return res.results[0]["out"].astype(np.float32)

